# revision 10
# baseline (speedup 1.0000x reference)
"""Trainium2 Bass kernel for a DP-GAT layer (dense masked attention).

Computes, for x:[B,N,D], A_shape:[N,N] (0/1 adjacency), q,k,v:[D,D]:
    Q = x@q ; K = x@k
    S = Q @ K^T / sqrt(D)
    W = exp(8*tanh(S/8)) * A_shape
    out = (W / W.sum(-1, keepdims=True)) @ x @ v

Sharding: rows of N split across 8 NeuronCores (1024 rows each), SPMD,
no collectives. Each core computes its row-block of scores in a
flash-attention-style fused loop and writes its row-block of the
output. Host scatters inputs / gathers outputs.

The kernel is ScalarE(ACT)-bound: tanh and exp must each touch every
score element (33.6M per core) and only the ACT engine has
transcendentals, so the floor is ~2 x 262k cycles/partition. The whole
design keeps ACT at ~100% duty:

  - The small projections K^T=(x@k)^T, Q^T, x@v (1.6% of FLOPs) are
    precomputed on the host (fp32, rounded to fp16), freeing the PE and
    DVE from prep work and freeing 2 PSUM banks.
  - The [N, RB] adjacency row-block streams from HBM as fp8-e4m3 (0/1
    is exact): half the fp16 traffic (33.6MB/core vs 67). It streams
    (pool-paced, one 256KB strip per group) rather than sitting
    resident in SBUF: a resident mask's 13MB dependency-free startup
    DMA burst trips the chip's power governor into a low p-state that
    slows EVERY engine ~20-40% for the whole run (measured: 563us ->
    799us). Pool-paced streaming keeps sustained DMA at a gentle
    ~70 GB/s.
  - tanh writes fp16 (ACT rate is dtype-independent; halves SBUF
    traffic), exp is fp16->fp16.
  - PSUM: 4 banks score group + 2x2 banks double-buffered PV
    accumulator, so the end-of-chunk normalize never stalls the next
    chunk. PE start=True clears a full PSUM bank, so each acc's two
    banks are zeroed by two full-bank dummy matmuls and all real PV
    matmuls accumulate with start=False (col 128 = rowsum via ones
    column of xv).

Device-side flow (per core, per batch), groups of 4 key-tiles:
    S^T  = KT_tile^T @ QT_chunk      -> PSUM [128, 4, 512] fp32 (PE)
    u    = tanh(S^T / (8*sqrt(D)))   -> SBUF fp16 (ACT, scale fused)
    w    = exp(8*u)                  -> SBUF fp16 (ACT, scale fused)
    p    = w * mask_tile(fp8)        -> SBUF fp16 (DVE)
    acc[i,0:129] += p_slice^T @ xv   -> PSUM (PE)
    out = acc[:, :128] * (1/acc[:, 128])  -> DMA to DRAM (DVE)
"""

import math
import sys
from contextlib import ExitStack

import numpy as np

try:
    import concourse.bass as bass  # noqa: F401
except ImportError:  # pragma: no cover
    sys.path.insert(0, "/opt/trn_rl_repo")
    import concourse.bass as bass  # noqa: F401

import concourse.mybir as mybir
import concourse.tile as tile
from concourse import bacc
from concourse.bass_utils import run_bass_kernel_spmd

F32 = mybir.dt.float32
F16 = mybir.dt.float16
F8 = mybir.dt.float8e4

B, N, D = 4, 8192, 128
NCORES = 8
RB = N // NCORES  # query rows per core

IC = 512          # query-row chunk (free dim of score matmuls)
NIC = RB // IC    # i-chunks per core
JG = 4            # key 128-tiles per score group
NJT = N // 128    # key tiles total
NG = NJT // JG    # groups per i-chunk


def build_program():
    nc = bacc.Bacc("TRN2", target_bir_lowering=False, debug=False)

    # host-precomputed: kt=(x@k)^T, qt=(x@q)^T row-block, xv=x@v (+ones col)
    kt_d = nc.dram_tensor("kt", [B, D, N], F16, kind="ExternalInput").ap()
    qt_d = nc.dram_tensor("qt", [B, D, RB], F16, kind="ExternalInput").ap()
    # [key-in-tile, key-tile, col] so each partition's DMA run is contiguous
    xv_d = nc.dram_tensor("xv", [B, 128, NJT, 130], F16, kind="ExternalInput").ap()
    # [partition, i-chunk, group, JG*IC]: one contiguous 2KB run per
    # partition per group strip (512B runs measurably aggravate the HBM
    # activity throttle)
    mask_d = nc.dram_tensor("maskT", [128, NIC, NG, JG * IC], F8, kind="ExternalInput").ap()
    out_d = nc.dram_tensor("out", [B, RB, D], F32, kind="ExternalOutput").ap()

    tanh_scale = 1.0 / (8.0 * math.sqrt(float(D)))

    with tile.TileContext(nc) as tc, ExitStack() as ctx:
        consts = ctx.enter_context(tc.tile_pool(name="consts", bufs=1))
        kt_pool = ctx.enter_context(tc.tile_pool(name="kt", bufs=2))
        qt_pool = ctx.enter_context(tc.tile_pool(name="qt", bufs=2))
        xv_pool = ctx.enter_context(tc.tile_pool(name="xv", bufs=2))
        m_pool = ctx.enter_context(tc.tile_pool(name="m", bufs=3))
        u_pool = ctx.enter_context(tc.tile_pool(name="u", bufs=3))
        w_pool = ctx.enter_context(tc.tile_pool(name="w", bufs=3))
        p_pool = ctx.enter_context(tc.tile_pool(name="p", bufs=3))
        ob_pool = ctx.enter_context(tc.tile_pool(name="ob", bufs=4))
        rs_pool = ctx.enter_context(tc.tile_pool(name="rs", bufs=4))
        st_ps = ctx.enter_context(tc.tile_pool(name="st_ps", bufs=1, space="PSUM"))
        acc_ps = ctx.enter_context(tc.tile_pool(name="acc_ps", bufs=2, space="PSUM"))

        zeros = consts.tile([128, 512], F16)
        nc.vector.memset(zeros[:], 0.0)

        tiles = {}  # b -> (kt, qt, xv)

        def load_batch(b):
            """Issue DMAs for batch b's kt/qt/xv (4 strips each so the
            first score group's deps land early)."""
            kt = kt_pool.tile([128, N], F16)
            qt = qt_pool.tile([128, RB], F16)
            xv = xv_pool.tile([128, NJT, 130], F16)
            tiles[b] = (kt, qt, xv)
            nc.sync.dma_start(qt[:], qt_d[b])
            for s in range(4):
                ks = N // 4
                nc.sync.dma_start(
                    kt[:, s * ks : (s + 1) * ks], kt_d[b][:, s * ks : (s + 1) * ks]
                )
                ts = NJT // 4
                nc.sync.dma_start(
                    xv[:, s * ts : (s + 1) * ts, :],
                    xv_d[b][:, s * ts : (s + 1) * ts, :],
                )

        def zero_acc(acc):
            # PE start=True clears the WHOLE PSUM bank, so the two acc
            # slots sharing a bank are zeroed by one full-bank dummy
            # matmul; all real PV matmuls accumulate with start=False.
            for hb in range(2):
                nc.tensor.matmul(
                    acc[:, hb * 512 : (hb + 1) * 512],
                    zeros[:, 0:128], zeros[:],
                    start=True, stop=False, skip_group_check=True,
                )

        def group(b, ic, g, acc):
            kt, qt, xv = tiles[b]
            stp = st_ps.tile([128, JG, IC], F32)
            for j in range(JG):
                nc.tensor.matmul(
                    stp[:, j],
                    kt[:, (g * JG + j) * 128 : (g * JG + j + 1) * 128],
                    qt[:, ic * IC : (ic + 1) * IC],
                    start=True, stop=True,
                )
            if g == 0:
                # placed after the first score matmuls so the PE can issue
                # them immediately at chunk start
                zero_acc(acc)
            u = u_pool.tile([128, JG, IC], F16)
            nc.scalar.activation(
                u[:], stp[:], mybir.ActivationFunctionType.Tanh, scale=tanh_scale
            )
            w = w_pool.tile([128, JG, IC], F16)
            nc.scalar.activation(
                w[:], u[:], mybir.ActivationFunctionType.Exp, scale=8.0
            )
            m = m_pool.tile([128, JG, IC], F8)
            nc.sync.dma_start(m[:].rearrange("p j i -> p (j i)"), mask_d[:, ic, g])
            p = p_pool.tile([128, JG, IC], F16)
            nc.vector.tensor_mul(p[:], w[:], m[:])
            for j in range(JG):
                for s in range(IC // 128):
                    nc.tensor.matmul(
                        acc[:, s * 256 : s * 256 + 129],
                        p[:, j, s * 128 : (s + 1) * 128],
                        xv[:, g * JG + j, 0:129],
                        start=False,
                        stop=(g == NG - 1 and j == JG - 1),
                        skip_group_check=True,
                    )

        load_batch(0)
        for b in range(B):
            for ic in range(NIC):
                if ic == NIC - 1 and b + 1 < B:
                    load_batch(b + 1)
                acc = acc_ps.tile([128, 1024], F32)
                for g in range(NG):
                    group(b, ic, g, acc)
                for s in range(IC // 128):
                    rs = rs_pool.tile([128, 1], F32)
                    nc.vector.reciprocal(rs[:], acc[:, s * 256 + 128 : s * 256 + 129])
                    ob = ob_pool.tile([128, 128], F32)
                    nc.vector.tensor_scalar_mul(
                        ob[:], acc[:, s * 256 : s * 256 + 128], rs[:]
                    )
                    nc.sync.dma_start(
                        out_d[b, ic * IC + s * 128 : ic * IC + (s + 1) * 128, :],
                        ob[:],
                    )

    nc.compile()
    return nc


_CACHED_NC = None


def _get_program():
    global _CACHED_NC
    if _CACHED_NC is None:
        _CACHED_NC = build_program()
    return _CACHED_NC


def make_in_maps(x, A_shape, q, k, v):
    x32 = np.ascontiguousarray(x, dtype=np.float32).reshape(-1, D)
    K = (x32 @ np.asarray(k, np.float32)).reshape(B, N, D)
    Q = (x32 @ np.asarray(q, np.float32)).reshape(B, N, D)
    XV = (x32 @ np.asarray(v, np.float32)).reshape(B, N, D)

    kt = np.ascontiguousarray(K.transpose(0, 2, 1)).astype(np.float16)  # [B,D,N]
    xv = np.zeros((B, N, 130), np.float16)
    xv[:, :, :128] = XV.astype(np.float16)
    xv[:, :, 128] = 1.0
    # [B, key-in-tile, key-tile, col]: contiguous per-partition DMA runs
    xv = np.ascontiguousarray(xv.reshape(B, NJT, 128, 130).transpose(0, 2, 1, 3))

    f8 = np.dtype(mybir.dt.np(F8))
    A32 = np.asarray(A_shape, np.float32)
    in_maps = []
    for c in range(NCORES):
        r0 = c * RB
        qt = np.ascontiguousarray(
            Q[:, r0 : r0 + RB, :].transpose(0, 2, 1)
        ).astype(np.float16)
        # maskT [N, RB] -> [key-in-tile, i-chunk, group, JG*IC]
        maskT = np.ascontiguousarray(
            A32[r0 : r0 + RB, :].T
            .reshape(NG, JG, 128, NIC, IC)
            .transpose(2, 3, 0, 1, 4)
            .reshape(128, NIC, NG, JG * IC)
        ).astype(f8)
        in_maps.append({"kt": kt, "qt": qt, "xv": xv, "maskT": maskT})
    return in_maps


def kernel(x, A_shape, q, k, v):
    nc = _get_program()
    in_maps = make_in_maps(x, A_shape, q, k, v)
    res = run_bass_kernel_spmd(nc, in_maps, list(range(NCORES)))
    out = np.concatenate([res.results[c]["out"] for c in range(NCORES)], axis=1)
    return out.astype(np.float32)
